# revision 74
# baseline (speedup 1.0000x reference)
"""Distributed single-head causal attention for TRN2 (8 NeuronCores).

Problem: x[B=4, T=4096, C=768], Wq/Wk/Wv[H=64, C] ->
  out[b,t,:] = softmax(causal(q k^T * C^-0.05)) @ v   (single head)

Sharding: core ci = (batch b = ci//2, interleave half h = ci%2). Each core
computes k/v for its whole batch and attention for the 16 q-tiles {2m+h}.

All 8 cores run ONE graph (uniform SPMD); every per-core difference is
carried in per-core DRAM inputs (a per-core COLUMN PERMUTATION of x and the
0/1 P-mask), never in instruction-stream structure or AP offsets.

v2 design (from the v1 trace: PE idle at start, HAM half-clock windows,
exp-paced attention, DMA issue overhead, serialized tail):
  - x is streamed ONCE as 24 [128,1024] f16 chunks (no separate xq stream).
    Host permutes columns per-core so block li = [my 4 q-tiles | partner 4
    tiles]; the q projection reads the fixed [:, 0:512] slice of the same
    chunks the kv projection consumes.  DMA drops 9.2MB -> 6.6MB and all
    chunk DMAs are emitted up front on both queues.
  - exact-causal trim: within the diagonal block, chunk d (0..7) only
    multiplies q-tiles >= tl_min(d); S matmul, exp and O matmul all shrink
    together.  Diagonal chunks pair (d, d+4) -> equal widths -> one strided
    exp per pair ([128,2,w] AP).
  - causal masking = one 128-wide 0/1 bf16 multiply on P per diagonal chunk
    (gpsimd/Pool engine), replacing 256-wide f32 PSUM adds on DVE.
  - warmup burst shrunk 16->6 matmuls (exp-LUT preload kept).
  - tail: O^T transposes land in disjoint slices of one PSUM tile (no
    serialize), one batched output DMA per li ([512,64] each).
  - lazy drains: each li's exp-gated tail O-pairs + normalize are emitted
    between the NEXT li's S-pairs, so the in-order PE never stalls on them.
  - each block's kv groups ride its own attention window (pairs 1-4), not
    the previous one's (whose chunks may not have landed).
Precision: f16 q/k/x/W, bf16 P/V, f32 elsewhere.  No row-max subtraction
(masked scores stay in [-53,51]; exp exact in f32).

Measured on trn2 (neuron-profile, whole NEFF): ~82-83us per core (v1
baseline 92.8-93.8us), rel err 2.28e-3 (gate 2e-2).  Fixed framework
overhead inside the measured window: ~6.8us preamble-to-first-DMA +
~7.5us postamble (8-way engine barrier + ~51 semaphore resets).
Aggregate DMA is ~210GB/s shared across all queues (one AXI port), so
the first ~15us are DMA-bound: scratch 'filler' matmuls pad the PE there
to keep the HAM clock governor at full speed (it demotes the core to
half clock after ~2-3us of PE idleness and needs ~4-5us of sustained
activity to promote).
"""

import sys

for _p in ("/opt/trn_rl_repo",):
    if _p not in sys.path:
        sys.path.insert(0, _p)

import ml_dtypes
import numpy as np

import concourse.bass as bass  # noqa: F401  (registers engine classes)
import concourse.tile as tile
from concourse import bacc, mybir
from concourse.bass_utils import run_bass_kernel_spmd

B, T, C, H = 4, 4096, 768, 64
NCORES = 8
SCALE = float(C ** (-0.05))
CCH = C // 128          # 6 contraction chunks
NSC = T // 128          # 32 s-chunks
TQ = T // 2             # 2048 q columns per core
NWARM = 16              # warmup matmuls (PE clock ramp)

F32 = mybir.dt.float32
BF16 = mybir.dt.bfloat16
F16 = mybir.dt.float16
EXP = mybir.ActivationFunctionType.Exp

_CACHE: dict = {}

# diagonal-chunk trim: chunk d of a block only hits q-tiles >= TLMIN[d]
TLMIN = [0, 1, 2, 3, 0, 1, 2, 3]


def _install_ntff_hook():
    """Provide antenv.axon_hooks if the image lacks it, so
    run_bass_kernel_spmd(trace=True) can capture NTFF profiles under axon."""
    try:
        from antenv.axon_hooks import get_axon_ntff_profile_hook  # noqa: F401
        return  # already present
    except ImportError:
        pass
    import contextlib
    import ctypes
    import types

    so_path = "/opt/axon/libaxon_pjrt.so"
    mod = types.ModuleType("antenv.axon_hooks")
    _state = {"hook": None}
    mod.set_axon_ntff_profile_hook = lambda h: _state.__setitem__("hook", h)
    mod.get_axon_ntff_profile_hook = lambda: _state["hook"]
    try:
        lib = ctypes.CDLL(so_path)
        if hasattr(lib, "axon_start_nrt_profile"):
            lib.axon_start_nrt_profile.argtypes = [
                ctypes.POINTER(ctypes.c_int64), ctypes.c_size_t]
            lib.axon_start_nrt_profile.restype = ctypes.c_int64
            lib.axon_stop_nrt_profile.argtypes = [ctypes.c_char_p]
            lib.axon_stop_nrt_profile.restype = ctypes.c_int64

            @contextlib.contextmanager
            def _hook(output_dir, device_ids):
                import jax
                jax.devices()
                if device_ids:
                    ids = (ctypes.c_int64 * len(device_ids))(*device_ids)
                    rc = lib.axon_start_nrt_profile(ids, len(device_ids))
                else:
                    rc = lib.axon_start_nrt_profile(None, 0)
                if rc != 0:
                    raise RuntimeError(f"axon_start_nrt_profile rc={rc}")
                try:
                    yield
                finally:
                    n = lib.axon_stop_nrt_profile(str(output_dir).encode())
                    print(f"profile: {n} file(s) written to {output_dir}")

            _state["hook"] = _hook
    except OSError:
        pass
    import antenv
    sys.modules["antenv.axon_hooks"] = mod
    antenv.axon_hooks = mod


_install_ntff_hook()


def _build_graph():
    nc = bacc.Bacc("TRN2", target_bir_lowering=False, debug=False,
                   num_devices=NCORES)

    xt_d = nc.dram_tensor("xt", [C, T], F16, kind="ExternalInput")
    wkv_d = nc.dram_tensor("wkv", [C, 128], F16, kind="ExternalInput")
    wq_d = nc.dram_tensor("wq", [C, 128], F16, kind="ExternalInput")
    m01_d = nc.dram_tensor("m01", [128, 2 * 128], BF16, kind="ExternalInput")
    id_d = nc.dram_tensor("ident", [128, 128], F16, kind="ExternalInput")
    id32_d = nc.dram_tensor("ident32", [H + 1, H + 1], F32, kind="ExternalInput")
    out_d = nc.dram_tensor("out", [TQ, H], F32, kind="ExternalOutput")

    with tile.TileContext(nc) as tc:
        with (
            tc.tile_pool(name="consts", bufs=1) as consts,
            tc.tile_pool(name="persist", bufs=1) as persist,
            tc.tile_pool(name="ptile", bufs=6) as ptile,
            tc.tile_pool(name="opost", bufs=4) as opost,
            tc.tile_pool(name="pskv", bufs=1, space="PSUM") as pskv,
            tc.tile_pool(name="psv", bufs=1, space="PSUM") as psv,
            tc.tile_pool(name="pss", bufs=2, space="PSUM") as pss,
            tc.tile_pool(name="pso", bufs=2, space="PSUM") as pso,
        ):
            # ---- constants + x stream, in strict consumption order across
            # both DMA queues (aggregate DMA bw is ~210GB/s shared; the
            # startup is DMA-bound so queue order = need order).
            wq_t = consts.tile([128, CCH * 128], F16, tag="wq", name="wq_t")
            wkv_t = consts.tile([128, CCH * 128], F16, tag="wkv", name="wkv_t")
            id_t = consts.tile([128, 128], F16, tag="ident", name="id_t")
            id32_t = consts.tile([H + 1, H + 1], F32, tag="id32", name="id32_t")
            m01_t = consts.tile([128, 2 * 128], BF16, tag="m01", name="m01_t")

            nc.sync.dma_start(
                wq_t[:].rearrange("p (c m) -> p c m", c=CCH),
                wq_d.ap().rearrange("(c p) m -> p c m", p=128))
            xs = [[None] * CCH for _ in range(4)]

            def dma_chunk(li, c, q):
                t_ = persist.tile([128, 1024], F16, tag=f"xs{li}_{c}",
                                  name=f"xs{li}_{c}")
                q.dma_start(t_[:], xt_d.ap()[c * 128:(c + 1) * 128,
                                             li * 1024:(li + 1) * 1024])
                xs[li][c] = t_

            for li in range(4):
                for c in (0, 2, 4):
                    dma_chunk(li, c, nc.sync)
            dma_chunk(0, 1, nc.gpsimd)
            dma_chunk(0, 3, nc.gpsimd)
            dma_chunk(0, 5, nc.gpsimd)
            nc.gpsimd.dma_start(
                wkv_t[:].rearrange("p (c m) -> p c m", c=CCH),
                wkv_d.ap().rearrange("(c p) m -> p c m", p=128))
            nc.gpsimd.dma_start(id_t[:], id_d.ap()[:, :])
            nc.gpsimd.dma_start(m01_t[:], m01_d.ap()[:, :])
            nc.gpsimd.dma_start(id32_t[:], id32_d.ap()[:, :])
            for li in range(1, 4):
                for c in (1, 3, 5):
                    dma_chunk(li, c, nc.gpsimd)

            # ---- warmup: preload Exp LUT + wake the PE clock while DMAs
            # stream (writes scratch nothing reads).  fill() emits keep-alive
            # matmuls: the HAM clock governor demotes the core to half speed
            # after ~2us of PE idleness and needs ~5us of sustained activity
            # to promote, so the DMA-bound start is padded with scratch work.
            wsc = persist.tile([128, 512], F16, tag="wsc", name="wsc")
            nc.vector.memset(wsc[:], 0.25)
            wact = persist.tile([128, 64], F32, tag="wact", name="wact")
            nc.vector.memset(wact[:], 0.5)
            nc.scalar.activation(wact[:], wact[:], EXP, scale=SCALE)

            def fill(n):
                for _ in range(n):
                    wps = pss.tile([128, 2, 512], F32, tag="s", name="wps")
                    nc.tensor.matmul(wps[:, 0, :], lhsT=wsc[:, 0:128],
                                     rhs=wsc[:], start=True, stop=True)

            fill(NWARM)

            # ---- persistent intermediates ----
            kvt = persist.tile([128, T], F16, tag="kvt", name="kvt")
            qt = persist.tile([128, TQ], F16, tag="qt", name="qt")
            vaug = persist.tile([128, NSC * (H + 1)], BF16, tag="vaug",
                               name="vaug")
            # ones column of V_aug (accumulates the softmax denominator):
            # single strided memset over all 32 chunks
            nc.vector.memset(vaug.rearrange(
                "p (sc w) -> p sc w", w=H + 1)[:, :, H:H + 1], 1.0)

            # ---- projection phase bodies (per 1024-col block li) ----
            # FPC[li] scratch matmuls after each chunk of the q group pad
            # the DMA-paced cadence (~1.2us/chunk) so the PE never idles
            FPC = [3, 1, 0, 0]

            def emit_proj_q(li):
                psq = pskv.tile([128, 512], F32, tag="kv", name=f"psq{li}")
                for c in range(CCH):
                    nc.tensor.matmul(psq[:], lhsT=wq_t[:, c * 128:(c + 1) * 128],
                                     rhs=xs[li][c][:, 0:512],
                                     start=(c == 0), stop=(c == CCH - 1))
                    if c < CCH - 1:
                        fill(FPC[li])
                # rows 64:127 are zero (wq zero-padded) -> copying all 128
                # rows keeps qt's v-rows zero for the fused S matmul
                nc.vector.tensor_copy(qt[:, li * 512:(li + 1) * 512], psq[:])
                fill(1)

            def emit_proj_kv(li, half):
                pkv = pskv.tile([128, 512], F32, tag="kv", name=f"pkv{li}_{half}")
                lo = half * 512
                for c in range(CCH):
                    nc.tensor.matmul(pkv[:], lhsT=wkv_t[:, c * 128:(c + 1) * 128],
                                     rhs=xs[li][c][:, lo:lo + 512],
                                     start=(c == 0), stop=(c == CCH - 1))
                base = li * 1024 + lo
                nc.vector.tensor_copy(kvt[:, base:base + 512], pkv[:])

            def emit_vtrans(li, half):
                # V rows of the 4 new kv chunks -> vaug (plus ones col, set
                # once above).  All 4 full-chunk transposes land in disjoint
                # slices of one psv tile -> no serialization.
                pv = psv.tile([128, 512], F16, tag="v", name=f"pv{li}_{half}")
                for k in range(4):
                    sc = li * 8 + half * 4 + k
                    nc.tensor.transpose(
                        pv[:, k * 128:(k + 1) * 128],
                        kvt[:, sc * 128:(sc + 1) * 128], id_t[:])
                    nc.vector.tensor_copy(
                        vaug[:, sc * (H + 1):sc * (H + 1) + H],
                        pv[:, k * 128 + 64:k * 128 + 128])

            # ---- attention for q-block li (yields between pair groups so
            # the caller can weave proj(li+1) work into exp-wait bubbles).
            # pending: leftover exp-gated drain/normalize steps of att(li-1),
            # emitted here between li's S-pairs so the in-order PE never
            # stalls on them (their exps are long done by now); this att's
            # own leftovers are appended to pending_out unless li==3.
            def emit_att(li, pending, pending_out):
                po = pso.tile([H + 1, 512], F32, tag="o", name=f"po{li}")
                nfull = 8 * li          # full-width s-chunks (earlier blocks)
                state = {}
                plist = []              # (p, chunk_a, chunk_b, off, w, diag)
                for p in range(nfull // 2):
                    plist.append((p, 2 * p, 2 * p + 1, 0, 512, False))
                for d in range(4):
                    plist.append((nfull // 2 + d, nfull + d, nfull + d + 4,
                                  d * 128, 512 - d * 128, True))
                npairs = len(plist)
                first_chunk = plist[0][1]
                last_chunk = plist[-1][2]

                def emit_spair(i):
                    p, ca, cb, off, w, diag = plist[i]
                    ps_ = pss.tile([128, 2, 512], F32, tag="s",
                                   name=f"ps{li}_{p}")
                    pp = ptile.tile([128, 2, 512], BF16, tag="p",
                                    name=f"pp{li}_{p}")
                    for j, si in enumerate((ca, cb)):
                        nc.tensor.matmul(
                            ps_[:, j, off:512],
                            lhsT=kvt[:, si * 128:(si + 1) * 128],
                            rhs=qt[:, li * 512 + off:(li + 1) * 512],
                            start=True, stop=True)
                    if off == 0:
                        # contiguous [128,1024] AP (the strided 3D form costs
                        # the ACT engine ~100ns extra per instruction)
                        nc.scalar.activation(
                            pp.rearrange("p a b -> p (a b)"),
                            ps_.rearrange("p a b -> p (a b)"),
                            EXP, scale=SCALE)
                    else:
                        nc.scalar.activation(pp[:, :, off:512],
                                             ps_[:, :, off:512],
                                             EXP, scale=SCALE)
                    if diag:
                        # j=0: own-parity chunk -> causal triangle; j=1:
                        # partner chunk -> all-0 (h=0) / all-1 (h=1) block.
                        # li=3's muls gate the final drain: DVE is idle
                        # there and ~2x faster per op than Pool.  (All-DVE
                        # measured 16us WORSE: Pool's parallelism matters.)
                        eng = nc.vector if li == 3 else nc.gpsimd
                        for j in range(2):
                            eng.tensor_mul(
                                pp[:, j, off:off + 128],
                                pp[:, j, off:off + 128],
                                m01_t[:, j * 128:(j + 1) * 128])
                    state[i] = pp

                def emit_opair(i):
                    p, ca, cb, off, w, diag = plist[i]
                    pp = state.pop(i)
                    for j, si in enumerate((ca, cb)):
                        nc.tensor.matmul(
                            po[:, off:512],
                            lhsT=vaug[:, si * (H + 1):(si + 1) * (H + 1)],
                            rhs=pp[:, j, off:512],
                            start=(si == first_chunk), stop=(si == last_chunk),
                            skip_group_check=True)

                LA = 3
                on = opost.tile([128, 4 * H], F32, tag="on", name=f"on{li}")
                for i in range(npairs):
                    emit_spair(i)
                    if li == 0:
                        fill(1)
                    if pending:
                        pending.pop(0)()
                    if i >= LA:
                        emit_opair(i - LA)
                    yield
                steps = [lambda i=i: emit_opair(i)
                         for i in range(npairs - 3, npairs)]
                steps += [lambda k=k: emit_norm_k(li, po, k, on)
                          for k in range(4)]
                if li < 3:
                    pending_out.extend(steps)
                    yield
                    return
                # final drain: po column-tile k's last writer is diag
                # opair k, so norm tile k can interleave right behind it
                # (norm0's columns are complete before the drain starts)
                emit_norm_k(li, po, 0, on)
                for i in range(npairs - 3, npairs):
                    emit_opair(i)
                    emit_norm_k(li, po, i - (npairs - 4), on)
                yield

            # normalize + output of one 128-t tile: PSUM->SBUF copy,
            # transpose (alternating psv / the idle pskv bank so tiles
            # k,k+1 don't serialize on one bank's accumulation group),
            # 1/l scale, and a half-size out-DMA every second tile
            def emit_norm_k(li, po, k, on):
                osb = opost.tile([H + 1, 128], F32, tag="osb",
                                 name=f"osb{li}_{k}")
                nc.vector.tensor_copy(osb[:], po[:, k * 128:(k + 1) * 128])
                pool = psv if k % 2 == 0 else pskv
                pot = pool.tile([128, H + 1], F32,
                                tag=("v" if k % 2 == 0 else "kv"),
                                name=f"pot{li}_{k}")
                ptk = pot[:, 0:H + 1]
                nc.tensor.transpose(ptk, osb[:], id32_t[:])
                linv = opost.tile([128, 1], F32, tag="linv",
                                  name=f"linv{li}_{k}")
                nc.vector.reciprocal(linv[:], ptk[:, H:H + 1])
                nc.vector.tensor_scalar_mul(
                    on[:, k * H:(k + 1) * H], ptk[:, 0:H], linv[:])
                if k % 2 == 1:
                    nc.sync.dma_start(
                        out_d.ap()[li * 512 + (k - 1) * 128:
                                   li * 512 + (k + 1) * 128, :].rearrange(
                            "(k2 p) m -> p k2 m", p=128),
                        on[:, (k - 1) * H:(k + 1) * H].rearrange(
                            "p (k2 m) -> p k2 m", k2=2))

            # ---- schedule ----
            # att(li)'s own kv groups ride its first pairs: the q block is
            # the only hard prerequisite of att(li)'s full pairs, so kv(li)
            # can be computed DURING att(li) (its diag pairs come 5th+).
            # This keeps kv groups off att(li-1)'s window (where block-li
            # chunks may not have landed -> in-order PE stall) and gives the
            # exp-bound att windows exp-free PE work to chew on.
            def emit_gkv(li):
                emit_proj_kv(li, 0)
                yield
                emit_vtrans(li, 0)
                yield
                emit_proj_kv(li, 1)
                yield
                emit_vtrans(li, 1)
                yield

            emit_proj_q(0)
            for _ in emit_gkv(0):
                pass
            pending = []
            gkv = iter(())
            for li in range(4):
                pending_out = []
                att = emit_att(li, pending, pending_out)
                npairs_li = 4 * li + 4
                i = 0
                for _ in att:
                    i += 1
                    if 1 <= i <= 4:
                        next(gkv, None)        # kv groups of block li
                    if i == npairs_li and li < 3:
                        # qt(li+1) must be ready before att(li+1) pair 0;
                        # kv(li+1) rides att(li+1)'s own first pairs
                        emit_proj_q(li + 1)
                        gkv = emit_gkv(li + 1)
                for step in pending:   # anything the weave didn't consume
                    step()
                pending = pending_out
            for _ in gkv:
                pass

    nc.compile()
    return nc


def _host_inputs(x, Wq, Wk, Wv):
    """Build the 8 per-core input maps from the full problem inputs."""
    wkv = np.ascontiguousarray(
        np.concatenate([Wk.T, Wv.T], axis=1).astype(np.float16))  # [C, 128]
    wq = np.ascontiguousarray(np.concatenate(
        [Wq.T.astype(np.float16), np.zeros((C, 64), np.float16)], axis=1))
    ident = np.eye(128, dtype=np.float16)
    ident32 = np.eye(H + 1, dtype=np.float32)
    tri01 = (np.arange(128)[:, None] <= np.arange(128)[None, :])

    in_maps = []
    for ci in range(NCORES):
        b, h = divmod(ci, 2)
        # permuted column order: block li = [4 own q-tiles | 4 partner tiles]
        perm = []
        for li in range(4):
            for k in range(4):
                g = 8 * li + 2 * k + h
                perm.append(np.arange(g * 128, (g + 1) * 128))
            for k in range(4):
                g = 8 * li + 2 * k + (1 - h)
                perm.append(np.arange(g * 128, (g + 1) * 128))
        perm = np.concatenate(perm)
        xt = np.ascontiguousarray(x[b].T.astype(np.float16)[:, perm])  # [C,T]
        # 0/1 P-mask rows: d<4 -> causal triangle (s<=t keeps); d>=4 ->
        # all-zero for h=0 (partner chunk d-4 sits one tile above the
        # diagonal), all-one for h=1 (one tile below)
        m01 = np.empty((128, 2 * 128), np.float32)
        m01[:, 0:128] = tri01
        m01[:, 128:256] = 0.0 if h == 0 else 1.0
        m01 = np.ascontiguousarray(m01).astype(ml_dtypes.bfloat16)
        in_maps.append({
            "xt": xt, "wkv": wkv, "wq": wq,
            "m01": m01, "ident": ident, "ident32": ident32,
        })
    return in_maps


def _run(x, Wq, Wk, Wv, trace=False, trace_cores=None):
    if "nc" not in _CACHE:
        _CACHE["nc"] = _build_graph()
    nc = _CACHE["nc"]
    in_maps = _host_inputs(np.asarray(x), np.asarray(Wq),
                           np.asarray(Wk), np.asarray(Wv))
    res = run_bass_kernel_spmd(nc, in_maps, core_ids=list(range(NCORES)),
                               trace=trace, trace_cores=trace_cores)
    out = np.empty((B, T, H), np.float32)
    for ci in range(NCORES):
        b, h = divmod(ci, 2)
        core_out = np.asarray(res.results[ci]["out"])            # [TQ, H]
        for m in range(16):
            g = 2 * m + h
            out[b, g * 128:(g + 1) * 128, :] = \
                core_out[m * 128:(m + 1) * 128, :]
    return out, res


def kernel(x, Wq, Wk, Wv):
    out, _ = _run(x, Wq, Wk, Wv, trace=False)
    return out


# revision 75
# speedup vs baseline: 1.0126x; 1.0126x over previous
"""Distributed single-head causal attention for TRN2 (8 NeuronCores).

Problem: x[B=4, T=4096, C=768], Wq/Wk/Wv[H=64, C] ->
  out[b,t,:] = softmax(causal(q k^T * C^-0.05)) @ v   (single head)

Sharding: core ci = (batch b = ci//2, interleave half h = ci%2). Each core
computes k/v for its whole batch and attention for the 16 q-tiles {2m+h}.

All 8 cores run ONE graph (uniform SPMD); every per-core difference is
carried in per-core DRAM inputs (a per-core COLUMN PERMUTATION of x and the
0/1 P-mask), never in instruction-stream structure or AP offsets.

v2 design (from the v1 trace: PE idle at start, HAM half-clock windows,
exp-paced attention, DMA issue overhead, serialized tail):
  - x is streamed ONCE as 24 [128,1024] f16 chunks (no separate xq stream).
    Host permutes columns per-core so block li = [my 4 q-tiles | partner 4
    tiles]; the q projection reads the fixed [:, 0:512] slice of the same
    chunks the kv projection consumes.  DMA drops 9.2MB -> 6.6MB and all
    chunk DMAs are emitted up front on both queues.
  - exact-causal trim: within the diagonal block, chunk d (0..7) only
    multiplies q-tiles >= tl_min(d); S matmul, exp and O matmul all shrink
    together.  Diagonal chunks pair (d, d+4) -> equal widths -> one strided
    exp per pair ([128,2,w] AP).
  - causal masking = one 128-wide 0/1 bf16 multiply on P per diagonal chunk
    (gpsimd/Pool engine), replacing 256-wide f32 PSUM adds on DVE.
  - warmup burst shrunk 16->6 matmuls (exp-LUT preload kept).
  - tail: O^T transposes land in disjoint slices of one PSUM tile (no
    serialize), one batched output DMA per li ([512,64] each).
  - lazy drains: each li's exp-gated tail O-pairs + normalize are emitted
    between the NEXT li's S-pairs, so the in-order PE never stalls on them.
  - each block's kv groups ride its own attention window (pairs 1-4), not
    the previous one's (whose chunks may not have landed).
Precision: f16 q/k/x/W, bf16 P/V, f32 elsewhere.  No row-max subtraction
(masked scores stay in [-53,51]; exp exact in f32).

Measured on trn2 (neuron-profile, whole NEFF): ~82-83us per core (v1
baseline 92.8-93.8us), rel err 2.28e-3 (gate 2e-2).  Fixed framework
overhead inside the measured window: ~6.8us preamble-to-first-DMA +
~7.5us postamble (8-way engine barrier + ~51 semaphore resets).
Aggregate DMA is ~210GB/s shared across all queues (one AXI port), so
the first ~15us are DMA-bound: scratch 'filler' matmuls pad the PE there
to keep the HAM clock governor at full speed (it demotes the core to
half clock after ~2-3us of PE idleness and needs ~4-5us of sustained
activity to promote).
"""

import sys

for _p in ("/opt/trn_rl_repo",):
    if _p not in sys.path:
        sys.path.insert(0, _p)

import ml_dtypes
import numpy as np

import concourse.bass as bass  # noqa: F401  (registers engine classes)
import concourse.tile as tile
from concourse import bacc, mybir
from concourse.bass_utils import run_bass_kernel_spmd

B, T, C, H = 4, 4096, 768, 64
NCORES = 8
SCALE = float(C ** (-0.05))
CCH = C // 128          # 6 contraction chunks
NSC = T // 128          # 32 s-chunks
TQ = T // 2             # 2048 q columns per core
NWARM = 16              # warmup matmuls (PE clock ramp)

F32 = mybir.dt.float32
BF16 = mybir.dt.bfloat16
F16 = mybir.dt.float16
EXP = mybir.ActivationFunctionType.Exp

_CACHE: dict = {}

# diagonal-chunk trim: chunk d of a block only hits q-tiles >= TLMIN[d]
TLMIN = [0, 1, 2, 3, 0, 1, 2, 3]


def _install_ntff_hook():
    """Provide antenv.axon_hooks if the image lacks it, so
    run_bass_kernel_spmd(trace=True) can capture NTFF profiles under axon."""
    try:
        from antenv.axon_hooks import get_axon_ntff_profile_hook  # noqa: F401
        return  # already present
    except ImportError:
        pass
    import contextlib
    import ctypes
    import types

    so_path = "/opt/axon/libaxon_pjrt.so"
    mod = types.ModuleType("antenv.axon_hooks")
    _state = {"hook": None}
    mod.set_axon_ntff_profile_hook = lambda h: _state.__setitem__("hook", h)
    mod.get_axon_ntff_profile_hook = lambda: _state["hook"]
    try:
        lib = ctypes.CDLL(so_path)
        if hasattr(lib, "axon_start_nrt_profile"):
            lib.axon_start_nrt_profile.argtypes = [
                ctypes.POINTER(ctypes.c_int64), ctypes.c_size_t]
            lib.axon_start_nrt_profile.restype = ctypes.c_int64
            lib.axon_stop_nrt_profile.argtypes = [ctypes.c_char_p]
            lib.axon_stop_nrt_profile.restype = ctypes.c_int64

            @contextlib.contextmanager
            def _hook(output_dir, device_ids):
                import jax
                jax.devices()
                if device_ids:
                    ids = (ctypes.c_int64 * len(device_ids))(*device_ids)
                    rc = lib.axon_start_nrt_profile(ids, len(device_ids))
                else:
                    rc = lib.axon_start_nrt_profile(None, 0)
                if rc != 0:
                    raise RuntimeError(f"axon_start_nrt_profile rc={rc}")
                try:
                    yield
                finally:
                    n = lib.axon_stop_nrt_profile(str(output_dir).encode())
                    print(f"profile: {n} file(s) written to {output_dir}")

            _state["hook"] = _hook
    except OSError:
        pass
    import antenv
    sys.modules["antenv.axon_hooks"] = mod
    antenv.axon_hooks = mod


_install_ntff_hook()


def _build_graph():
    nc = bacc.Bacc("TRN2", target_bir_lowering=False, debug=False,
                   num_devices=NCORES)

    xt_d = nc.dram_tensor("xt", [C, T], F16, kind="ExternalInput")
    wkv_d = nc.dram_tensor("wkv", [C, 128], F16, kind="ExternalInput")
    wq_d = nc.dram_tensor("wq", [C, 128], F16, kind="ExternalInput")
    m01_d = nc.dram_tensor("m01", [128, 2 * 128], BF16, kind="ExternalInput")
    id_d = nc.dram_tensor("ident", [128, 128], F16, kind="ExternalInput")
    id32_d = nc.dram_tensor("ident32", [H + 1, H + 1], F32, kind="ExternalInput")
    out_d = nc.dram_tensor("out", [TQ, H], F32, kind="ExternalOutput")

    with tile.TileContext(nc) as tc:
        with (
            tc.tile_pool(name="consts", bufs=1) as consts,
            tc.tile_pool(name="persist", bufs=1) as persist,
            tc.tile_pool(name="ptile", bufs=6) as ptile,
            tc.tile_pool(name="opost", bufs=4) as opost,
            tc.tile_pool(name="pskv", bufs=1, space="PSUM") as pskv,
            tc.tile_pool(name="psv", bufs=1, space="PSUM") as psv,
            tc.tile_pool(name="pss", bufs=2, space="PSUM") as pss,
            tc.tile_pool(name="pso", bufs=2, space="PSUM") as pso,
        ):
            # ---- constants + x stream, in strict consumption order across
            # both DMA queues (aggregate DMA bw is ~210GB/s shared; the
            # startup is DMA-bound so queue order = need order).
            wq_t = consts.tile([128, CCH * 128], F16, tag="wq", name="wq_t")
            wkv_t = consts.tile([128, CCH * 128], F16, tag="wkv", name="wkv_t")
            id_t = consts.tile([128, 128], F16, tag="ident", name="id_t")
            id32_t = consts.tile([H + 1, H + 1], F32, tag="id32", name="id32_t")
            m01_t = consts.tile([128, 2 * 128], BF16, tag="m01", name="m01_t")

            nc.sync.dma_start(
                wq_t[:].rearrange("p (c m) -> p c m", c=CCH),
                wq_d.ap().rearrange("(c p) m -> p c m", p=128))
            xs = [[None] * CCH for _ in range(4)]

            def dma_chunk(li, c, q):
                t_ = persist.tile([128, 1024], F16, tag=f"xs{li}_{c}",
                                  name=f"xs{li}_{c}")
                q.dma_start(t_[:], xt_d.ap()[c * 128:(c + 1) * 128,
                                             li * 1024:(li + 1) * 1024])
                xs[li][c] = t_

            for li in range(4):
                for c in (0, 2, 4):
                    dma_chunk(li, c, nc.sync)
            dma_chunk(0, 1, nc.gpsimd)
            dma_chunk(0, 3, nc.gpsimd)
            dma_chunk(0, 5, nc.gpsimd)
            nc.gpsimd.dma_start(
                wkv_t[:].rearrange("p (c m) -> p c m", c=CCH),
                wkv_d.ap().rearrange("(c p) m -> p c m", p=128))
            nc.gpsimd.dma_start(id_t[:], id_d.ap()[:, :])
            nc.gpsimd.dma_start(m01_t[:], m01_d.ap()[:, :])
            nc.gpsimd.dma_start(id32_t[:], id32_d.ap()[:, :])
            for li in range(1, 4):
                for c in (1, 3, 5):
                    dma_chunk(li, c, nc.gpsimd)

            # ---- warmup: preload Exp LUT + wake the PE clock while DMAs
            # stream (writes scratch nothing reads).  fill() emits keep-alive
            # matmuls: the HAM clock governor demotes the core to half speed
            # after ~2us of PE idleness and needs ~5us of sustained activity
            # to promote, so the DMA-bound start is padded with scratch work.
            wsc = persist.tile([128, 512], F16, tag="wsc", name="wsc")
            nc.vector.memset(wsc[:], 0.25)
            wact = persist.tile([128, 64], F32, tag="wact", name="wact")
            nc.vector.memset(wact[:], 0.5)
            nc.scalar.activation(wact[:], wact[:], EXP, scale=SCALE)

            def fill(n):
                for _ in range(n):
                    wps = pss.tile([128, 2, 512], F32, tag="s", name="wps")
                    nc.tensor.matmul(wps[:, 0, :], lhsT=wsc[:, 0:128],
                                     rhs=wsc[:], start=True, stop=True)

            fill(NWARM)

            # ---- persistent intermediates ----
            kvt = persist.tile([128, T], F16, tag="kvt", name="kvt")
            qt = persist.tile([128, TQ], F16, tag="qt", name="qt")
            vaug = persist.tile([128, NSC * (H + 1)], BF16, tag="vaug",
                               name="vaug")
            # ones column of V_aug (accumulates the softmax denominator):
            # single strided memset over all 32 chunks
            nc.vector.memset(vaug.rearrange(
                "p (sc w) -> p sc w", w=H + 1)[:, :, H:H + 1], 1.0)

            # ---- projection phase bodies (per 1024-col block li) ----
            # FPC[li] scratch matmuls after each chunk of the q group pad
            # the DMA-paced cadence (~1.2us/chunk) so the PE never idles
            FPC = [3, 1, 0, 0]

            def emit_proj_q(li):
                psq = pskv.tile([128, 512], F32, tag="kv", name=f"psq{li}")
                for c in range(CCH):
                    nc.tensor.matmul(psq[:], lhsT=wq_t[:, c * 128:(c + 1) * 128],
                                     rhs=xs[li][c][:, 0:512],
                                     start=(c == 0), stop=(c == CCH - 1))
                    if c < CCH - 1:
                        fill(FPC[li])
                # rows 64:127 are zero (wq zero-padded) -> copying all 128
                # rows keeps qt's v-rows zero for the fused S matmul
                nc.vector.tensor_copy(qt[:, li * 512:(li + 1) * 512], psq[:])
                fill(1)

            def emit_proj_kv(li, half):
                pkv = pskv.tile([128, 512], F32, tag="kv", name=f"pkv{li}_{half}")
                lo = half * 512
                for c in range(CCH):
                    nc.tensor.matmul(pkv[:], lhsT=wkv_t[:, c * 128:(c + 1) * 128],
                                     rhs=xs[li][c][:, lo:lo + 512],
                                     start=(c == 0), stop=(c == CCH - 1))
                base = li * 1024 + lo
                nc.vector.tensor_copy(kvt[:, base:base + 512], pkv[:])

            def emit_vtrans(li, half):
                # V rows of the 4 new kv chunks -> vaug (plus ones col, set
                # once above).  All 4 full-chunk transposes land in disjoint
                # slices of one psv tile -> no serialization.
                pv = psv.tile([128, 512], F16, tag="v", name=f"pv{li}_{half}")
                for k in range(4):
                    sc = li * 8 + half * 4 + k
                    nc.tensor.transpose(
                        pv[:, k * 128:(k + 1) * 128],
                        kvt[:, sc * 128:(sc + 1) * 128], id_t[:])
                    nc.vector.tensor_copy(
                        vaug[:, sc * (H + 1):sc * (H + 1) + H],
                        pv[:, k * 128 + 64:k * 128 + 128])

            # ---- attention for q-block li (yields between pair groups so
            # the caller can weave proj(li+1) work into exp-wait bubbles).
            # pending: leftover exp-gated drain/normalize steps of att(li-1),
            # emitted here between li's S-pairs so the in-order PE never
            # stalls on them (their exps are long done by now); this att's
            # own leftovers are appended to pending_out unless li==3.
            def emit_att(li, pending, pending_out):
                po = pso.tile([H + 1, 512], F32, tag="o", name=f"po{li}")
                nfull = 8 * li          # full-width s-chunks (earlier blocks)
                state = {}
                plist = []              # (p, chunk_a, chunk_b, off, w, diag)
                for p in range(nfull // 2):
                    plist.append((p, 2 * p, 2 * p + 1, 0, 512, False))
                for d in range(4):
                    plist.append((nfull // 2 + d, nfull + d, nfull + d + 4,
                                  d * 128, 512 - d * 128, True))
                npairs = len(plist)
                first_chunk = plist[0][1]
                last_chunk = plist[-1][2]

                def emit_spair(i):
                    p, ca, cb, off, w, diag = plist[i]
                    ps_ = pss.tile([128, 2, 512], F32, tag="s",
                                   name=f"ps{li}_{p}")
                    pp = ptile.tile([128, 2, 512], BF16, tag="p",
                                    name=f"pp{li}_{p}")
                    for j, si in enumerate((ca, cb)):
                        nc.tensor.matmul(
                            ps_[:, j, off:512],
                            lhsT=kvt[:, si * 128:(si + 1) * 128],
                            rhs=qt[:, li * 512 + off:(li + 1) * 512],
                            start=True, stop=True)
                    if off == 0:
                        # contiguous [128,1024] AP (the strided 3D form costs
                        # the ACT engine ~100ns extra per instruction)
                        nc.scalar.activation(
                            pp.rearrange("p a b -> p (a b)"),
                            ps_.rearrange("p a b -> p (a b)"),
                            EXP, scale=SCALE)
                    else:
                        nc.scalar.activation(pp[:, :, off:512],
                                             ps_[:, :, off:512],
                                             EXP, scale=SCALE)
                    if diag:
                        # j=0: own-parity chunk -> causal triangle; j=1:
                        # partner chunk -> all-0 (h=0) / all-1 (h=1) block.
                        # li=3's muls gate the final drain: DVE is idle
                        # there and ~2x faster per op than Pool.  (All-DVE
                        # measured 16us WORSE: Pool's parallelism matters.)
                        eng = nc.vector if li == 3 else nc.gpsimd
                        for j in range(2):
                            eng.tensor_mul(
                                pp[:, j, off:off + 128],
                                pp[:, j, off:off + 128],
                                m01_t[:, j * 128:(j + 1) * 128])
                    state[i] = pp

                def emit_opair(i):
                    p, ca, cb, off, w, diag = plist[i]
                    pp = state.pop(i)
                    for j, si in enumerate((ca, cb)):
                        nc.tensor.matmul(
                            po[:, off:512],
                            lhsT=vaug[:, si * (H + 1):(si + 1) * (H + 1)],
                            rhs=pp[:, j, off:512],
                            start=(si == first_chunk), stop=(si == last_chunk),
                            skip_group_check=True)

                LA = 3
                on = opost.tile([128, 4 * H], F32, tag="on", name=f"on{li}")
                for i in range(npairs):
                    emit_spair(i)
                    if li == 0:
                        fill(1)
                    if pending:
                        pending.pop(0)()
                    if i >= LA:
                        emit_opair(i - LA)
                    yield
                steps = [lambda i=i: emit_opair(i)
                         for i in range(npairs - 3, npairs)]
                steps += [lambda k=k: emit_norm_k(li, po, k, on)
                          for k in range(4)]
                if li < 3:
                    pending_out.extend(steps)
                    yield
                    return
                # no fillers here: since the lazy-drain restructure the
                # last exps finish well before these steps, so scratch
                # matmuls would sit serially in the tail.  (Interleaving
                # norm_k behind each drain opair measured no better.)
                for s in steps:
                    s()
                yield

            # normalize + output of one 128-t tile: PSUM->SBUF copy,
            # transpose (alternating psv / the idle pskv bank so tiles
            # k,k+1 don't serialize on one bank's accumulation group),
            # 1/l scale, and a half-size out-DMA every second tile
            def emit_norm_k(li, po, k, on):
                osb = opost.tile([H + 1, 128], F32, tag="osb",
                                 name=f"osb{li}_{k}")
                nc.vector.tensor_copy(osb[:], po[:, k * 128:(k + 1) * 128])
                pool = psv if k % 2 == 0 else pskv
                pot = pool.tile([128, H + 1], F32,
                                tag=("v" if k % 2 == 0 else "kv"),
                                name=f"pot{li}_{k}")
                ptk = pot[:, 0:H + 1]
                nc.tensor.transpose(ptk, osb[:], id32_t[:])
                linv = opost.tile([128, 1], F32, tag="linv",
                                  name=f"linv{li}_{k}")
                nc.vector.reciprocal(linv[:], ptk[:, H:H + 1])
                nc.vector.tensor_scalar_mul(
                    on[:, k * H:(k + 1) * H], ptk[:, 0:H], linv[:])
                if k % 2 == 1:
                    nc.sync.dma_start(
                        out_d.ap()[li * 512 + (k - 1) * 128:
                                   li * 512 + (k + 1) * 128, :].rearrange(
                            "(k2 p) m -> p k2 m", p=128),
                        on[:, (k - 1) * H:(k + 1) * H].rearrange(
                            "p (k2 m) -> p k2 m", k2=2))

            # ---- schedule ----
            # att(li)'s own kv groups ride its first pairs: the q block is
            # the only hard prerequisite of att(li)'s full pairs, so kv(li)
            # can be computed DURING att(li) (its diag pairs come 5th+).
            # This keeps kv groups off att(li-1)'s window (where block-li
            # chunks may not have landed -> in-order PE stall) and gives the
            # exp-bound att windows exp-free PE work to chew on.
            def emit_gkv(li):
                emit_proj_kv(li, 0)
                yield
                emit_vtrans(li, 0)
                yield
                emit_proj_kv(li, 1)
                yield
                emit_vtrans(li, 1)
                yield

            emit_proj_q(0)
            for _ in emit_gkv(0):
                pass
            pending = []
            gkv = iter(())
            for li in range(4):
                pending_out = []
                att = emit_att(li, pending, pending_out)
                npairs_li = 4 * li + 4
                i = 0
                for _ in att:
                    i += 1
                    if 1 <= i <= 4:
                        next(gkv, None)        # kv groups of block li
                    if i == npairs_li and li < 3:
                        # qt(li+1) must be ready before att(li+1) pair 0;
                        # kv(li+1) rides att(li+1)'s own first pairs
                        emit_proj_q(li + 1)
                        gkv = emit_gkv(li + 1)
                for step in pending:   # anything the weave didn't consume
                    step()
                pending = pending_out
            for _ in gkv:
                pass

    nc.compile()
    return nc


def _host_inputs(x, Wq, Wk, Wv):
    """Build the 8 per-core input maps from the full problem inputs."""
    wkv = np.ascontiguousarray(
        np.concatenate([Wk.T, Wv.T], axis=1).astype(np.float16))  # [C, 128]
    wq = np.ascontiguousarray(np.concatenate(
        [Wq.T.astype(np.float16), np.zeros((C, 64), np.float16)], axis=1))
    ident = np.eye(128, dtype=np.float16)
    ident32 = np.eye(H + 1, dtype=np.float32)
    tri01 = (np.arange(128)[:, None] <= np.arange(128)[None, :])

    in_maps = []
    for ci in range(NCORES):
        b, h = divmod(ci, 2)
        # permuted column order: block li = [4 own q-tiles | 4 partner tiles]
        perm = []
        for li in range(4):
            for k in range(4):
                g = 8 * li + 2 * k + h
                perm.append(np.arange(g * 128, (g + 1) * 128))
            for k in range(4):
                g = 8 * li + 2 * k + (1 - h)
                perm.append(np.arange(g * 128, (g + 1) * 128))
        perm = np.concatenate(perm)
        xt = np.ascontiguousarray(x[b].T.astype(np.float16)[:, perm])  # [C,T]
        # 0/1 P-mask rows: d<4 -> causal triangle (s<=t keeps); d>=4 ->
        # all-zero for h=0 (partner chunk d-4 sits one tile above the
        # diagonal), all-one for h=1 (one tile below)
        m01 = np.empty((128, 2 * 128), np.float32)
        m01[:, 0:128] = tri01
        m01[:, 128:256] = 0.0 if h == 0 else 1.0
        m01 = np.ascontiguousarray(m01).astype(ml_dtypes.bfloat16)
        in_maps.append({
            "xt": xt, "wkv": wkv, "wq": wq,
            "m01": m01, "ident": ident, "ident32": ident32,
        })
    return in_maps


def _run(x, Wq, Wk, Wv, trace=False, trace_cores=None):
    if "nc" not in _CACHE:
        _CACHE["nc"] = _build_graph()
    nc = _CACHE["nc"]
    in_maps = _host_inputs(np.asarray(x), np.asarray(Wq),
                           np.asarray(Wk), np.asarray(Wv))
    res = run_bass_kernel_spmd(nc, in_maps, core_ids=list(range(NCORES)),
                               trace=trace, trace_cores=trace_cores)
    out = np.empty((B, T, H), np.float32)
    for ci in range(NCORES):
        b, h = divmod(ci, 2)
        core_out = np.asarray(res.results[ci]["out"])            # [TQ, H]
        for m in range(16):
            g = 2 * m + h
            out[b, g * 128:(g + 1) * 128, :] = \
                core_out[m * 128:(m + 1) * 128, :]
    return out, res


def kernel(x, Wq, Wk, Wv):
    out, _ = _run(x, Wq, Wk, Wv, trace=False)
    return out
